# revision 2
# baseline (speedup 1.0000x reference)
"""MultiHeadDiffAttention TRN2 kernel, v2.

Sharding: 8 cores = 2 batches x 4 head-pairs. Core c handles batch c//4 and
heads {2g, 2g+1} where g = c%4. The 2 heads = 128 channels = exactly one
GroupNorm group, so GroupNorm is core-local. The final projection is computed
as a partial sum over the core's 128 channels; the host adds the 4 partials
per batch plus the output bias.

The execution environment charges a large, roughly flat cost per instruction
(engine type / operand size barely matter), so the structure minimizes total
instruction count:
  - post-build sync pass: drops same-engine waits that are provably redundant
    (the target instruction cannot be stalled while the waiter is runnable),
    which stops the BIR lowering from splitting them into standalone
    event-semaphore instructions
  - packed PE-transposes: 4 transposes share one PSUM bank, staged out with
    one copy instead of four
  - identity/ones/eps constants ride the single packed weights DMA
  - single packed output DMA
  - combine phase: one reciprocal + one partition_broadcast per head
  - GroupNorm via tensor_reduce + Square(accum_out) + partition_all_reduce
"""

import os
import sys

sys.path.insert(0, "/opt/trn_rl_repo")

import numpy as np

import concourse.bacc as bacc
import concourse.mybir as mybir
import concourse.tile as tile
from concourse.bass_utils import run_bass_kernel_spmd

B, S, D = 2, 2048, 512
H = 8
G = 4
HD = D // H          # 64
CH = 2 * HD          # 128 channels per core (one GroupNorm group)
LAMBDA_INIT = 0.2
EPS = 1e-5
N_CORES = 8

QB = 512             # query block (matmul N)
NQB = S // QB        # 4
KB = 128             # key block (matmul M)
NKB = S // KB        # 16
KG = 4               # key blocks per exp group ([128, 2048] PSUM tile)
NKG = NKB // KG      # 4
SB = 128             # seq block for v / final matmul
NSB = S // SB        # 16

F32 = mybir.dt.float32
F32R = mybir.dt.float32r

NW = 5               # q1,k1,q2,k2,v
WCOLS = NW * 512     # 2560
OWOFF = WCOLS        # owT at [2560, 3072)
CPOFF = WCOLS + D    # cp at [3072, 3078)
IDOFF = CPOFF + 6    # 128x128 identity at [3078, 3206)
ONEOFF = IDOFF + 128 # ones column
EPSOFF = ONEOFF + 1  # eps column
TOTCOLS = EPSOFF + 1

_CACHE = {}

ENGINE_SHORT = {
    "EngineType.PE": "PE",
    "EngineType.DVE": "DVE",
    "EngineType.Activation": "Activation",
    "EngineType.Pool": "Pool",
    "EngineType.SP": "SP",
}


def _optimize_sync(nc):
    """Drop provably-redundant same-engine waits pre-compile.

    Engines dispatch in order but execute out of a small wait queue, so a
    same-engine wait is a real dependency in general. It IS redundant when
    the waiting instruction's remaining waits imply every wait of the wait's
    target J: then J can never be stalled while I is runnable, and queue
    FIFO order makes J complete before I executes. Each dropped wait saves
    one standalone InstEventSemaphore in the lowered program (the ISA holds
    one wait per instruction; extras are split into event-sem instructions).

    Only semaphores exclusively incremented (sem-inc, +1) by non-DMA
    instructions of a single engine are considered — DMA completions update
    semaphores asynchronously and must be waited on.
    """
    blocks = []

    def collect(b):
        blocks.append(b)
        for sub in getattr(b, "blocks", []):
            collect(sub)

    for f in nc.m.functions:
        for b in f.blocks:
            collect(b)

    updaters = {}
    for b in blocks:
        for inst in b.instructions:
            si = inst.sync_info
            if si is None:
                continue
            for u in (si.on_update or []):
                nm = u.ant_name or str(u.id)
                updaters.setdefault(nm, set()).add(
                    (str(inst.engine), type(inst).__name__, u.update_mode))

    own = {}  # engine str -> set of own monotonic sem names
    for nm, ups in updaters.items():
        engines = {e for e, _, _ in ups}
        opcodes = {o for _, o, _ in ups}
        modes = {m for _, _, m in ups}
        if (len(engines) == 1 and modes == {"sem-inc"}
                and not (opcodes & {"InstDMACopy", "InstTriggerDma",
                                    "InstDmaTransposeAnt"})):
            eng = next(iter(engines))
            short = ENGINE_SHORT.get(eng)
            if short and nm.startswith(short + "_"):
                own.setdefault(eng, set()).add(nm)

    # per-sem program-order incrementer list (only +1-increment own sems)
    inc_list = {}   # sem name -> [instruction, ...] in program order
    ok_sem = set()
    for sems in own.values():
        ok_sem |= sems
    for b in blocks:
        for inst in b.instructions:
            si = inst.sync_info
            if si is None:
                continue
            for u in (si.on_update or []):
                nm = u.ant_name or str(u.id)
                if nm in ok_sem:
                    if u.update_value != 1:
                        ok_sem.discard(nm)
                    else:
                        inc_list.setdefault(nm, []).append(inst)

    def implied(wait, wait_set):
        nm = wait.ant_name or str(wait.id)
        for w in wait_set:
            if ((w.ant_name or str(w.id)) == nm
                    and w.wait_mode == "sem-ge-imm"
                    and w.wait_value >= wait.wait_value):
                return True
        return False

    # Drop a same-engine wait (A >= a) on I when I's other waits imply all
    # waits of J = program-order a-th incrementer of A: then J can never be
    # stalled while I is runnable, and queue FIFO order makes J complete
    # before I executes — regardless of the engine's bypass window depth.
    n_dropped = 0
    for b in blocks:
        for inst in b.instructions:
            si = inst.sync_info
            if si is None or not si.on_wait or len(si.on_wait) < 2:
                continue
            mine = own.get(str(inst.engine), set())
            keep = list(si.on_wait)
            changed = True
            while changed and len(keep) > 1:
                changed = False
                for w in keep:
                    nm = w.ant_name or str(w.id)
                    if (w.sync_type != "semaphore"
                            or w.wait_mode != "sem-ge-imm"
                            or nm not in mine or nm not in ok_sem):
                        continue
                    lst = inc_list.get(nm, [])
                    a = w.wait_value
                    if not (1 <= a <= len(lst)):
                        continue
                    J = lst[a - 1]
                    jsi = J.sync_info
                    jwaits = list(jsi.on_wait or []) if jsi else []
                    rest = [x for x in keep if x is not w]
                    if all(x.sync_type == "semaphore"
                           and x.wait_mode == "sem-ge-imm"
                           and implied(x, rest) for x in jwaits):
                        keep = rest
                        n_dropped += 1
                        changed = True
                        break
            if len(keep) != len(si.on_wait):
                si.on_wait = keep
    if os.environ.get("KERNEL_DEBUG"):
        total = sum(len(b.instructions) for b in blocks)
        print(f"_optimize_sync: dropped {n_dropped} redundant same-engine "
              f"waits; {total} instructions pre-compile", file=sys.stderr)


def build_program(repeats=1):
    nc = bacc.Bacc("TRN2", target_bir_lowering=False, debug=False)

    # ---- external I/O (packed per-partition-contiguous host layouts) ----
    # xp[p, c, s] = x[b, s, 128c+p]
    d_xp = nc.declare_dram_parameter("xp", [128, 4 * S], F32, isOutput=False)
    # wp[p, :] = 5 weights (4 c-chunks x 128 out-ch each) | owT | cp
    d_wp = nc.declare_dram_parameter("wp", [128, TOTCOLS], F32, isOutput=False)
    # yp[p, sb, d] = y_part[128*sb+p, d]
    d_y = nc.declare_dram_parameter("y_part", [SB, NSB * D], F32, isOutput=True)

    with tile.TileContext(nc) as tc:
     for _rep in range(repeats):
      with tc.tile_pool(name="main", bufs=1) as main:
        # ---- constants / packed inputs (identity/ones/eps ride the
        # same DMA as the weights) ----
        wp = main.tile([128, TOTCOLS], F32R, tag="wp")
        nc.sync.dma_start(out=wp, in_=d_wp.ap().bitcast(F32R))
        ones = wp[:, ONEOFF:ONEOFF + 1].bitcast(F32)
        eps_t = wp[:, EPSOFF:EPSOFF + 1].bitcast(F32)
        ident = wp[:, IDOFF:IDOFF + 128]
        owT = wp[:, OWOFF:OWOFF + D]
        cp = wp[:, CPOFF:CPOFF + 6].bitcast(F32)
        k1b, k2b = cp[:, 0:1], cp[:, 1:2]
        gnw, gnb = cp[:, 2:3], cp[:, 3:4]
        neglam = cp[:, 4:6]

        def wchunk(w, c):
            off = (w * 4 + c) * CH
            return wp[:, off:off + CH]

        WIDX = {"q1": 0, "k1": 1, "q2": 2, "k2": 3, "v": 4}

        qk = {}
        va = main.tile([SB, 2 * NSB, HD + 1], F32R, tag="va")
        # U[h]: [65, attn, S] exp-weight sums staged from PSUM
        U = {h: main.tile([HD + 1, 2, S], F32, tag=f"U{h}", name=f"U{h}")
             for h in (0, 1)}
        # pre-place the ut ring buffers in main's region (below the
        # projection-phase scopes) so exps inherit no scope-exit joins
        for _i in range(2):
            _ = main.tile([128, KG * QB], F32R, tag=f"ut{_i}",
                          name=f"utpin{_i}")

        # ---- phase 1+2: projections & attention (qk tiles scoped) ----
        with tc.tile_pool(name="ppool", bufs=1) as ppool:
            # ---- projections: qT/kT [128, 2048] channel-major ----
            with (
                tc.tile_pool(name="xtp", bufs=1) as xt_pool,
                tc.tile_pool(name="pj", bufs=1, space="PSUM") as pj_pool,
                tc.tile_pool(name="pv", bufs=2, space="PSUM") as pv_pool,
            ):
                xt = xt_pool.tile([128, 4, S], F32R, tag="xt")
                nc.sync.dma_start(
                    out=xt,
                    in_=d_xp.ap().bitcast(F32R).rearrange(
                        "p (c s) -> p c s", c=4))

                for w, bias in (("k1", k1b), ("q1", None), ("k2", k2b),
                                ("q2", None), ("v", None)):
                    dst = ppool.tile([CH, S], F32R, tag=w)
                    qk[w] = dst
                    ps = pj_pool.tile([CH, 4 * QB], F32, tag="pj", name="pj")
                    for qb in range(NQB):
                        for c in range(4):
                            nc.tensor.matmul(
                                ps[:, qb * QB:(qb + 1) * QB],
                                wchunk(WIDX[w], c),
                                xt[:, c, qb * QB:(qb + 1) * QB],
                                start=(c == 0),
                                stop=(c == 3),
                            )
                    if bias is not None:
                        nc.vector.tensor_scalar_add(dst, ps, bias)
                    else:
                        nc.vector.tensor_copy(dst, ps)

                # ---- va: token-major v [s, hd] + ones column, slot 2*sb+h.
                # 4 PE transposes pack one PSUM bank -> 1 staging copy. ----
                vT = qk["v"]
                nc.vector.tensor_copy(
                    va[:, :, HD:HD + 1],
                    ones.to_broadcast((SB, 2 * NSB, 1)))
                for g in range(NSB // 4):
                    pst = pv_pool.tile([SB, 4, SB], F32R, tag="pv", name="pv")
                    for j in range(4):
                        sb = 4 * g + j
                        nc.tensor.matmul(
                            pst[:, j, :],
                            vT[:, sb * SB:(sb + 1) * SB],
                            ident,
                            is_transpose=True,
                            start=(j == 0), stop=(j == 3),
                            skip_group_check=(j != 0),
                        )
                    nc.vector.tensor_copy(
                        va[:, 8 * g:8 * (g + 1), 0:HD],
                        pst.rearrange("p j (h m) -> p (j h) m", h=2))

            # ---- attention ----
            with (
                tc.tile_pool(name="sc", bufs=1, space="PSUM") as sc_pool,
                tc.tile_pool(name="av", bufs=1, space="PSUM") as av_pool,
            ):
                for attn in (1, 2):
                    qT, kT = qk[f"q{attn}"], qk[f"k{attn}"]
                    for h in (0, 1):
                        av = av_pool.tile([HD + 1, NQB * QB], F32,
                                          tag="av", name="av")
                        for qb in range(NQB):
                            for kg in range(NKG):
                                sct = sc_pool.tile([128, KG * QB], F32,
                                                   tag="sc", name="sc")
                                for j in range(KG):
                                    kb = kg * KG + j
                                    nc.tensor.matmul(
                                        sct[:, j * QB:(j + 1) * QB],
                                        kT[h * HD:(h + 1) * HD,
                                           kb * KB:(kb + 1) * KB],
                                        qT[h * HD:(h + 1) * HD,
                                           qb * QB:(qb + 1) * QB],
                                        start=True, stop=True,
                                    )
                                nexp = ((attn - 1) * 2 + h) * 16 + qb * NKG + kg
                                ut = main.tile(
                                    [128, KG * QB], F32R,
                                    tag=f"ut{nexp % 2}",
                                    name=f"ut{nexp}")
                                nc.scalar.activation(
                                    out=ut, in_=sct,
                                    func=mybir.ActivationFunctionType.Exp,
                                    scale=1.0 / (HD ** 0.5),
                                )
                                for j in range(KG):
                                    kb = kg * KG + j
                                    nc.tensor.matmul(
                                        av[:, qb * QB:(qb + 1) * QB],
                                        va[:, 2 * kb + h, :],
                                        ut[:, j * QB:(j + 1) * QB],
                                        start=(kb == 0),
                                        stop=(kb == NKB - 1),
                                    )
                        # stage U (incl. denominator row 64) to SBUF
                        nc.vector.tensor_copy(U[h][:, attn - 1, :], av)

        # ---- phase 3: combine, GroupNorm, final projection ----
        with tc.tile_pool(name="post", bufs=1) as post:
            oT = post.tile([CH, S], F32, tag="oT")
            xn = post.tile([CH, S], F32R, tag="xn")

            # combine: o = U1/r1 - lam * U2/r2 (one recip/bcast per head)
            for h in (0, 1):
                hs = slice(h * HD, (h + 1) * HD)
                rr = post.tile([1, 2, S], F32, tag="rr", name="rr")
                nc.vector.reciprocal(out=rr, in_=U[h][HD:HD + 1, :, :])
                rb = post.tile([HD, 2, S], F32, tag="rb", name="rb")
                nc.gpsimd.partition_broadcast(rb, rr)
                t1 = post.tile([HD, S], F32, tag="t1", name="t1")
                nc.vector.tensor_mul(t1, U[h][0:HD, 0, :], rb[:, 0, :])
                t2 = post.tile([HD, S], F32, tag="t2", name="t2")
                nc.vector.scalar_tensor_tensor(
                    out=t2, in0=U[h][0:HD, 1, :],
                    scalar=neglam[0:HD, h:h + 1], in1=rb[:, 1, :],
                    op0=mybir.AluOpType.mult,
                    op1=mybir.AluOpType.mult,
                )
                nc.vector.tensor_add(oT[hs, :], t1, t2)

            # ---- GroupNorm (whole [128, 2048] is one group) ----
            if True:
                # per-partition [sum(o), sum(o^2)], then partition-sum on
                # gpsimd; Square's accum_out gives sum(o^2) for free
                s12 = post.tile([CH, 2], F32, tag="s12")
                nc.vector.tensor_reduce(
                    out=s12[:, 0:1], in_=oT, axis=mybir.AxisListType.X,
                    op=mybir.AluOpType.add)
                sqt = post.tile([CH, S], F32, tag="sqt")
                nc.scalar.activation(
                    out=sqt, in_=oT,
                    func=mybir.ActivationFunctionType.Square,
                    accum_out=s12[:, 1:2])
                import concourse.bass_isa as bass_isa
                st = post.tile([CH, 2], F32, tag="st")
                nc.gpsimd.partition_all_reduce(
                    st, s12, channels=CH, reduce_op=bass_isa.ReduceOp.add)
                mu_e2 = post.tile([CH, 2], F32, tag="mu_e2")
                nc.vector.tensor_scalar_mul(mu_e2, st, 1.0 / (CH * S))
                sqm = post.tile([CH, 1], F32, tag="sqm")
                nc.vector.tensor_mul(sqm, mu_e2[:, 0:1], mu_e2[:, 0:1])
                var = post.tile([CH, 1], F32, tag="var")
                nc.vector.tensor_sub(var, mu_e2[:, 1:2], sqm)
                std = post.tile([CH, 1], F32, tag="std")
                nc.scalar.activation(out=std, in_=var,
                                     func=mybir.ActivationFunctionType.Sqrt,
                                     bias=eps_t, scale=1.0)
                a_t = post.tile([CH, 1], F32, tag="a_t")
                nc.vector.reciprocal(out=a_t, in_=std)
                nc.vector.tensor_mul(a_t, a_t, gnw)
                amu = post.tile([CH, 1], F32, tag="amu")
                nc.vector.tensor_mul(amu, a_t, mu_e2[:, 0:1])
                b_t = post.tile([CH, 1], F32, tag="b_t")
                nc.vector.tensor_sub(b_t, gnb, amu)
                nc.vector.tensor_scalar(out=xn, in0=oT, scalar1=a_t,
                                        scalar2=b_t,
                                        op0=mybir.AluOpType.mult,
                                        op1=mybir.AluOpType.add)

            # ---- final projection partial: y = xn.T @ owT ----
            yt = post.tile([SB, NSB, D], F32, tag="yt")
            with tc.tile_pool(name="fin", bufs=2, space="PSUM") as fin_pool:
                half = NSB // 4
                for hf in range(4):
                    ps = fin_pool.tile([SB, half * D], F32, tag="fin",
                                       name="fin")
                    for i in range(half):
                        sb = hf * half + i
                        nc.tensor.matmul(
                            ps[:, i * D:(i + 1) * D],
                            xn[:, sb * SB:(sb + 1) * SB],
                            owT,
                            start=True, stop=True,
                        )
                    nc.vector.tensor_copy(
                        yt[:, hf * half:(hf + 1) * half, :],
                        ps.rearrange("p (i d) -> p i d", i=half))
            nc.sync.dma_start(
                out=d_y.ap().rearrange("p (sb d) -> p sb d", sb=NSB),
                in_=yt)

    if not os.environ.get("KERNEL_NOSYNC_OPT"):
        _optimize_sync(nc)
    nc.compile()
    return nc


def _shard_inputs(inputs):
    x = np.ascontiguousarray(inputs["x"], np.float32)
    lam = (np.exp(inputs["lambda_q1"] * inputs["lambda_k1"])
           - np.exp(inputs["lambda_q2"] * inputs["lambda_k2"])
           + LAMBDA_INIT).astype(np.float32).reshape(H)
    in_maps = []
    for c in range(N_CORES):
        b, g = divmod(c, 4)
        ch = slice(CH * g, CH * (g + 1))
        # xp[p, c, s] = x[b, s, 128c+p]
        xp = np.ascontiguousarray(
            x[b].T.reshape(4, 128, S).transpose(1, 0, 2).reshape(128, 4 * S))
        wlist = []
        for W in (inputs["Q1_w"], inputs["K1_w"], inputs["Q2_w"],
                  inputs["K2_w"], inputs["V_w"]):
            wT = np.asarray(W)[ch].T  # [512, 128]
            wlist.append(np.ascontiguousarray(
                wT.reshape(4, 128, CH).transpose(1, 0, 2).reshape(128, 512)))
        owT = np.ascontiguousarray(np.asarray(inputs["out_w"])[:, ch].T)
        cpcols = np.stack([
            np.asarray(inputs["K1_b"])[ch],
            np.asarray(inputs["K2_b"])[ch],
            np.asarray(inputs["gn_w"])[ch],
            np.asarray(inputs["gn_b"])[ch],
            np.full(CH, -lam[2 * g], np.float32),
            np.full(CH, -lam[2 * g + 1], np.float32),
        ], axis=1).astype(np.float32)
        extra = np.concatenate([
            np.eye(128, dtype=np.float32),
            np.ones((128, 1), np.float32),
            np.full((128, 1), EPS, np.float32),
        ], axis=1)
        wp = np.concatenate(wlist + [owT, cpcols, extra],
                            axis=1).astype(np.float32)
        in_maps.append({"xp": xp, "wp": np.ascontiguousarray(wp)})
    return in_maps


def kernel(**inputs):
    inputs = {k: np.asarray(v) for k, v in inputs.items()}
    if "nc" not in _CACHE:
        _CACHE["nc"] = build_program()
    nc = _CACHE["nc"]
    in_maps = _shard_inputs(inputs)
    res = run_bass_kernel_spmd(nc, in_maps, list(range(N_CORES)))
    out_b = np.asarray(inputs["out_b"], np.float32)
    y = np.zeros((B, S, D), np.float32)
    for c in range(N_CORES):
        b = c // 4
        yp = res.results[c]["y_part"].astype(np.float32)
        y[b] += yp.reshape(SB, NSB, D).transpose(1, 0, 2).reshape(S, D)
    y += out_b[None, None, :]
    return y


# revision 4
# speedup vs baseline: 1.8446x; 1.8446x over previous
"""MultiHeadDiffAttention TRN2 kernel, v2.

Sharding: 8 cores = 2 batches x 4 head-pairs. Core c handles batch c//4 and
heads {2g, 2g+1} where g = c%4. The 2 heads = 128 channels = exactly one
GroupNorm group, so GroupNorm is core-local. The final projection is computed
as a partial sum over the core's 128 channels; the host adds the 4 partials
per batch plus the output bias.

The execution environment charges a large, roughly flat cost per instruction
(engine type / operand size barely matter), so the structure minimizes total
instruction count:
  - post-build sync pass: drops same-engine waits that are provably redundant
    (the target instruction cannot be stalled while the waiter is runnable),
    which stops the BIR lowering from splitting them into standalone
    event-semaphore instructions
  - packed PE-transposes: 4 transposes share one PSUM bank, staged out with
    one copy instead of four
  - identity/ones/eps constants ride the single packed weights DMA
  - single packed output DMA
  - combine phase: one reciprocal + one partition_broadcast per head
  - GroupNorm via tensor_reduce + Square(accum_out) + partition_all_reduce
"""

import os
import sys

sys.path.insert(0, "/opt/trn_rl_repo")

import numpy as np

import concourse.bacc as bacc
import concourse.mybir as mybir
import concourse.tile as tile
from concourse.bass_utils import run_bass_kernel_spmd

B, S, D = 2, 2048, 512
H = 8
G = 4
HD = D // H          # 64
CH = 2 * HD          # 128 channels per core (one GroupNorm group)
LAMBDA_INIT = 0.2
EPS = 1e-5
N_CORES = 8

QB = 512             # query block (matmul N)
NQB = S // QB        # 4
KB = 128             # key block (matmul M)
NKB = S // KB        # 16
KG = 4               # key blocks per exp group ([128, 2048] PSUM tile)
NKG = NKB // KG      # 4
SB = 128             # seq block for v / final matmul
NSB = S // SB        # 16

F32 = mybir.dt.float32
F32R = mybir.dt.float32r

NW = 5               # q1,k1,q2,k2,v
WCOLS = NW * 512     # 2560
OWOFF = WCOLS        # owT at [2560, 3072)
CPOFF = WCOLS + D    # cp at [3072, 3078)
IDOFF = CPOFF + 6    # 128x128 identity at [3078, 3206)
ONEOFF = IDOFF + 128 # ones column
EPSOFF = ONEOFF + 1  # eps column
TOTCOLS = EPSOFF + 1

_CACHE = {}

ENGINE_SHORT = {
    "EngineType.PE": "PE",
    "EngineType.DVE": "DVE",
    "EngineType.Activation": "Activation",
    "EngineType.Pool": "Pool",
    "EngineType.SP": "SP",
}


def _optimize_sync(nc):
    """Drop provably-redundant same-engine waits pre-compile.

    Engines dispatch in order but execute out of a small wait queue, so a
    same-engine wait is a real dependency in general. It IS redundant when
    the waiting instruction's remaining waits imply every wait of the wait's
    target J: then J can never be stalled while I is runnable, and queue
    FIFO order makes J complete before I executes. Each dropped wait saves
    one standalone InstEventSemaphore in the lowered program (the ISA holds
    one wait per instruction; extras are split into event-sem instructions).

    Only semaphores exclusively incremented (sem-inc, +1) by non-DMA
    instructions of a single engine are considered — DMA completions update
    semaphores asynchronously and must be waited on.
    """
    blocks = []

    def collect(b):
        blocks.append(b)
        for sub in getattr(b, "blocks", []):
            collect(sub)

    for f in nc.m.functions:
        for b in f.blocks:
            collect(b)

    updaters = {}
    for b in blocks:
        for inst in b.instructions:
            si = inst.sync_info
            if si is None:
                continue
            for u in (si.on_update or []):
                nm = u.ant_name or str(u.id)
                updaters.setdefault(nm, set()).add(
                    (str(inst.engine), type(inst).__name__, u.update_mode))

    own = {}  # engine str -> set of own monotonic sem names
    for nm, ups in updaters.items():
        engines = {e for e, _, _ in ups}
        opcodes = {o for _, o, _ in ups}
        modes = {m for _, _, m in ups}
        if (len(engines) == 1 and modes == {"sem-inc"}
                and not (opcodes & {"InstDMACopy", "InstTriggerDma",
                                    "InstDmaTransposeAnt"})):
            eng = next(iter(engines))
            short = ENGINE_SHORT.get(eng)
            if short and nm.startswith(short + "_"):
                own.setdefault(eng, set()).add(nm)

    # per-sem program-order incrementer list (only +1-increment own sems)
    inc_list = {}   # sem name -> [instruction, ...] in program order
    ok_sem = set()
    for sems in own.values():
        ok_sem |= sems
    for b in blocks:
        for inst in b.instructions:
            si = inst.sync_info
            if si is None:
                continue
            for u in (si.on_update or []):
                nm = u.ant_name or str(u.id)
                if nm in ok_sem:
                    if u.update_value != 1:
                        ok_sem.discard(nm)
                    else:
                        inc_list.setdefault(nm, []).append(inst)

    def implied(wait, wait_set):
        nm = wait.ant_name or str(wait.id)
        for w in wait_set:
            if ((w.ant_name or str(w.id)) == nm
                    and w.wait_mode == "sem-ge-imm"
                    and w.wait_value >= wait.wait_value):
                return True
        return False

    # Drop a same-engine wait (A >= a) on I when I's other waits imply all
    # waits of J = program-order a-th incrementer of A: then J can never be
    # stalled while I is runnable, and queue FIFO order makes J complete
    # before I executes — regardless of the engine's bypass window depth.
    n_dropped = 0
    for b in blocks:
        for inst in b.instructions:
            si = inst.sync_info
            if si is None or not si.on_wait or len(si.on_wait) < 2:
                continue
            mine = own.get(str(inst.engine), set())
            keep = list(si.on_wait)
            changed = True
            while changed and len(keep) > 1:
                changed = False
                for w in keep:
                    nm = w.ant_name or str(w.id)
                    if (w.sync_type != "semaphore"
                            or w.wait_mode != "sem-ge-imm"
                            or nm not in mine or nm not in ok_sem):
                        continue
                    lst = inc_list.get(nm, [])
                    a = w.wait_value
                    if not (1 <= a <= len(lst)):
                        continue
                    J = lst[a - 1]
                    jsi = J.sync_info
                    jwaits = list(jsi.on_wait or []) if jsi else []
                    rest = [x for x in keep if x is not w]
                    if all(x.sync_type == "semaphore"
                           and x.wait_mode == "sem-ge-imm"
                           and implied(x, rest) for x in jwaits):
                        keep = rest
                        n_dropped += 1
                        changed = True
                        break
            if len(keep) != len(si.on_wait):
                si.on_wait = keep
    if os.environ.get("KERNEL_DEBUG"):
        total = sum(len(b.instructions) for b in blocks)
        print(f"_optimize_sync: dropped {n_dropped} redundant same-engine "
              f"waits; {total} instructions pre-compile", file=sys.stderr)


def build_program(repeats=1):
    nc = bacc.Bacc("TRN2", target_bir_lowering=False, debug=False)

    # ---- external I/O (packed per-partition-contiguous host layouts) ----
    # xp[p, c, s] = x[b, s, 128c+p]
    d_xp = nc.declare_dram_parameter("xp", [128, 4 * S], F32, isOutput=False)
    # wp[p, :] = 5 weights (4 c-chunks x 128 out-ch each) | owT | cp
    d_wp = nc.declare_dram_parameter("wp", [128, TOTCOLS], F32, isOutput=False)
    # yp[p, sb, d] = y_part[128*sb+p, d]
    d_y = nc.declare_dram_parameter("y_part", [SB, NSB * D], F32, isOutput=True)

    with tile.TileContext(nc) as tc:
     for _rep in range(repeats):
      with tc.tile_pool(name="main", bufs=1) as main:
        # ---- constants / packed inputs (identity/ones/eps ride the
        # same DMA as the weights) ----
        wp = main.tile([128, TOTCOLS], F32R, tag="wp")
        nc.sync.dma_start(out=wp, in_=d_wp.ap().bitcast(F32R))
        ones = wp[:, ONEOFF:ONEOFF + 1].bitcast(F32)
        eps_t = wp[:, EPSOFF:EPSOFF + 1].bitcast(F32)
        ident = wp[:, IDOFF:IDOFF + 128]
        owT = wp[:, OWOFF:OWOFF + D]
        cp = wp[:, CPOFF:CPOFF + 6].bitcast(F32)
        k1b, k2b = cp[:, 0:1], cp[:, 1:2]
        gnw, gnb = cp[:, 2:3], cp[:, 3:4]
        neglam = cp[:, 4:6]

        def wchunk(w, c):
            off = (w * 4 + c) * CH
            return wp[:, off:off + CH]

        WIDX = {"q1": 0, "k1": 1, "q2": 2, "k2": 3, "v": 4}

        qk = {}
        va = main.tile([SB, 2 * NSB, HD + 1], F32R, tag="va")
        # U[h]: [65, attn, S] exp-weight sums staged from PSUM
        U = {h: main.tile([HD + 1, 2, S], F32, tag=f"U{h}", name=f"U{h}")
             for h in (0, 1)}
        # two long-lived exp-output buffers, alternated across kg groups;
        # plain tiles (no pool ring) so rewrites carry precise WAR/WAW deps
        uts = [main.tile([128, KG * QB], F32R, tag=f"ut{_i}",
                         name=f"ut{_i}") for _i in range(4)]

        # ---- phase 1+2: projections & attention (qk tiles scoped) ----
        with tc.tile_pool(name="ppool", bufs=1) as ppool:
            # ---- projections: qT/kT [128, 2048] channel-major ----
            with (
                tc.tile_pool(name="xtp", bufs=1) as xt_pool,
                tc.tile_pool(name="pj", bufs=1, space="PSUM") as pj_pool,
                tc.tile_pool(name="pv", bufs=2, space="PSUM") as pv_pool,
            ):
                xt = xt_pool.tile([128, 4, S], F32R, tag="xt")
                nc.sync.dma_start(
                    out=xt,
                    in_=d_xp.ap().bitcast(F32R).rearrange(
                        "p (c s) -> p c s", c=4))

                for w, bias in (("k1", k1b), ("q1", None), ("k2", k2b),
                                ("q2", None), ("v", None)):
                    dst = ppool.tile([CH, S], F32R, tag=w)
                    qk[w] = dst
                    ps = pj_pool.tile([CH, 4 * QB], F32, tag="pj", name="pj")
                    for qb in range(NQB):
                        for c in range(4):
                            nc.tensor.matmul(
                                ps[:, qb * QB:(qb + 1) * QB],
                                wchunk(WIDX[w], c),
                                xt[:, c, qb * QB:(qb + 1) * QB],
                                start=(c == 0),
                                stop=(c == 3),
                            )
                    if bias is not None:
                        nc.vector.tensor_scalar_add(dst, ps, bias)
                    else:
                        nc.vector.tensor_copy(dst, ps)

                # ---- va: token-major v [s, hd] + ones column, slot 2*sb+h.
                # 4 PE transposes pack one PSUM bank -> 1 staging copy. ----
                vT = qk["v"]
                nc.vector.tensor_copy(
                    va[:, :, HD:HD + 1],
                    ones.to_broadcast((SB, 2 * NSB, 1)))
                for g in range(NSB // 4):
                    pst = pv_pool.tile([SB, 4, SB], F32R, tag="pv", name="pv")
                    for j in range(4):
                        sb = 4 * g + j
                        nc.tensor.matmul(
                            pst[:, j, :],
                            vT[:, sb * SB:(sb + 1) * SB],
                            ident,
                            is_transpose=True,
                            start=(j == 0), stop=(j == 3),
                            skip_group_check=(j != 0),
                        )
                    nc.vector.tensor_copy(
                        va[:, 8 * g:8 * (g + 1), 0:HD],
                        pst.rearrange("p j (h m) -> p (j h) m", h=2))

            # ---- attention ----
            with (
                tc.tile_pool(name="sc", bufs=1, space="PSUM") as sc_pool,
                tc.tile_pool(name="av", bufs=1, space="PSUM") as av_pool,
            ):
                for attn in (1, 2):
                    qT, kT = qk[f"q{attn}"], qk[f"k{attn}"]
                    for h in (0, 1):
                        av = av_pool.tile([HD + 1, NQB * QB], F32,
                                          tag="av", name="av")
                        for qb in range(NQB):
                            for kg in range(NKG):
                                sct = sc_pool.tile([128, KG * QB], F32,
                                                   tag="sc", name="sc")
                                for j in range(KG):
                                    kb = kg * KG + j
                                    nc.tensor.matmul(
                                        sct[:, j * QB:(j + 1) * QB],
                                        kT[h * HD:(h + 1) * HD,
                                           kb * KB:(kb + 1) * KB],
                                        qT[h * HD:(h + 1) * HD,
                                           qb * QB:(qb + 1) * QB],
                                        start=True, stop=True,
                                    )
                                nexp = ((attn - 1) * 2 + h) * 16 + qb * NKG + kg
                                ut = uts[nexp % 4]
                                nc.scalar.activation(
                                    out=ut, in_=sct,
                                    func=mybir.ActivationFunctionType.Exp,
                                    scale=1.0 / (HD ** 0.5),
                                )
                                for j in range(KG):
                                    kb = kg * KG + j
                                    nc.tensor.matmul(
                                        av[:, qb * QB:(qb + 1) * QB],
                                        va[:, 2 * kb + h, :],
                                        ut[:, j * QB:(j + 1) * QB],
                                        start=(kb == 0),
                                        stop=(kb == NKB - 1),
                                    )
                        # stage U (incl. denominator row 64) to SBUF
                        nc.vector.tensor_copy(U[h][:, attn - 1, :], av)

        # ---- phase 3: combine, GroupNorm, final projection ----
        with tc.tile_pool(name="post", bufs=1) as post:
            oT = post.tile([CH, S], F32, tag="oT")
            xn = post.tile([CH, S], F32R, tag="xn")

            # combine: o = U1/r1 - lam * U2/r2 (one recip/bcast per head)
            for h in (0, 1):
                hs = slice(h * HD, (h + 1) * HD)
                rr = post.tile([1, 2, S], F32, tag="rr", name="rr")
                nc.vector.reciprocal(out=rr, in_=U[h][HD:HD + 1, :, :])
                rb = post.tile([HD, 2, S], F32, tag="rb", name="rb")
                nc.gpsimd.partition_broadcast(rb, rr)
                t1 = post.tile([HD, S], F32, tag="t1", name="t1")
                nc.vector.tensor_mul(t1, U[h][0:HD, 0, :], rb[:, 0, :])
                t2 = post.tile([HD, S], F32, tag="t2", name="t2")
                nc.vector.scalar_tensor_tensor(
                    out=t2, in0=U[h][0:HD, 1, :],
                    scalar=neglam[0:HD, h:h + 1], in1=rb[:, 1, :],
                    op0=mybir.AluOpType.mult,
                    op1=mybir.AluOpType.mult,
                )
                nc.vector.tensor_add(oT[hs, :], t1, t2)

            # ---- GroupNorm (whole [128, 2048] is one group) ----
            if True:
                # per-partition [sum(o), sum(o^2)], then partition-sum on
                # gpsimd; Square's accum_out gives sum(o^2) for free
                s12 = post.tile([CH, 2], F32, tag="s12")
                nc.vector.tensor_reduce(
                    out=s12[:, 0:1], in_=oT, axis=mybir.AxisListType.X,
                    op=mybir.AluOpType.add)
                sqt = post.tile([CH, S], F32, tag="sqt")
                nc.scalar.activation(
                    out=sqt, in_=oT,
                    func=mybir.ActivationFunctionType.Square,
                    accum_out=s12[:, 1:2])
                import concourse.bass_isa as bass_isa
                st = post.tile([CH, 2], F32, tag="st")
                nc.gpsimd.partition_all_reduce(
                    st, s12, channels=CH, reduce_op=bass_isa.ReduceOp.add)
                mu_e2 = post.tile([CH, 2], F32, tag="mu_e2")
                nc.vector.tensor_scalar_mul(mu_e2, st, 1.0 / (CH * S))
                sqm = post.tile([CH, 1], F32, tag="sqm")
                nc.vector.tensor_mul(sqm, mu_e2[:, 0:1], mu_e2[:, 0:1])
                var = post.tile([CH, 1], F32, tag="var")
                nc.vector.tensor_sub(var, mu_e2[:, 1:2], sqm)
                std = post.tile([CH, 1], F32, tag="std")
                nc.scalar.activation(out=std, in_=var,
                                     func=mybir.ActivationFunctionType.Sqrt,
                                     bias=eps_t, scale=1.0)
                a_t = post.tile([CH, 1], F32, tag="a_t")
                nc.vector.reciprocal(out=a_t, in_=std)
                nc.vector.tensor_mul(a_t, a_t, gnw)
                amu = post.tile([CH, 1], F32, tag="amu")
                nc.vector.tensor_mul(amu, a_t, mu_e2[:, 0:1])
                b_t = post.tile([CH, 1], F32, tag="b_t")
                nc.vector.tensor_sub(b_t, gnb, amu)
                nc.vector.tensor_scalar(out=xn, in0=oT, scalar1=a_t,
                                        scalar2=b_t,
                                        op0=mybir.AluOpType.mult,
                                        op1=mybir.AluOpType.add)

            # ---- final projection partial: y = xn.T @ owT ----
            yt = post.tile([SB, NSB, D], F32, tag="yt")
            with tc.tile_pool(name="fin", bufs=2, space="PSUM") as fin_pool:
                half = NSB // 4
                for hf in range(4):
                    ps = fin_pool.tile([SB, half * D], F32, tag="fin",
                                       name="fin")
                    for i in range(half):
                        sb = hf * half + i
                        nc.tensor.matmul(
                            ps[:, i * D:(i + 1) * D],
                            xn[:, sb * SB:(sb + 1) * SB],
                            owT,
                            start=True, stop=True,
                        )
                    nc.vector.tensor_copy(
                        yt[:, hf * half:(hf + 1) * half, :],
                        ps.rearrange("p (i d) -> p i d", i=half))
            nc.sync.dma_start(
                out=d_y.ap().rearrange("p (sb d) -> p sb d", sb=NSB),
                in_=yt)

    if not os.environ.get("KERNEL_NOSYNC_OPT"):
        _optimize_sync(nc)
    nc.compile()
    return nc


def _shard_inputs(inputs):
    x = np.ascontiguousarray(inputs["x"], np.float32)
    lam = (np.exp(inputs["lambda_q1"] * inputs["lambda_k1"])
           - np.exp(inputs["lambda_q2"] * inputs["lambda_k2"])
           + LAMBDA_INIT).astype(np.float32).reshape(H)
    in_maps = []
    for c in range(N_CORES):
        b, g = divmod(c, 4)
        ch = slice(CH * g, CH * (g + 1))
        # xp[p, c, s] = x[b, s, 128c+p]
        xp = np.ascontiguousarray(
            x[b].T.reshape(4, 128, S).transpose(1, 0, 2).reshape(128, 4 * S))
        wlist = []
        for W in (inputs["Q1_w"], inputs["K1_w"], inputs["Q2_w"],
                  inputs["K2_w"], inputs["V_w"]):
            wT = np.asarray(W)[ch].T  # [512, 128]
            wlist.append(np.ascontiguousarray(
                wT.reshape(4, 128, CH).transpose(1, 0, 2).reshape(128, 512)))
        owT = np.ascontiguousarray(np.asarray(inputs["out_w"])[:, ch].T)
        cpcols = np.stack([
            np.asarray(inputs["K1_b"])[ch],
            np.asarray(inputs["K2_b"])[ch],
            np.asarray(inputs["gn_w"])[ch],
            np.asarray(inputs["gn_b"])[ch],
            np.full(CH, -lam[2 * g], np.float32),
            np.full(CH, -lam[2 * g + 1], np.float32),
        ], axis=1).astype(np.float32)
        extra = np.concatenate([
            np.eye(128, dtype=np.float32),
            np.ones((128, 1), np.float32),
            np.full((128, 1), EPS, np.float32),
        ], axis=1)
        wp = np.concatenate(wlist + [owT, cpcols, extra],
                            axis=1).astype(np.float32)
        in_maps.append({"xp": xp, "wp": np.ascontiguousarray(wp)})
    return in_maps


def kernel(**inputs):
    inputs = {k: np.asarray(v) for k, v in inputs.items()}
    if "nc" not in _CACHE:
        _CACHE["nc"] = build_program()
    nc = _CACHE["nc"]
    in_maps = _shard_inputs(inputs)
    res = run_bass_kernel_spmd(nc, in_maps, list(range(N_CORES)))
    out_b = np.asarray(inputs["out_b"], np.float32)
    y = np.zeros((B, S, D), np.float32)
    for c in range(N_CORES):
        b = c // 4
        yp = res.results[c]["y_part"].astype(np.float32)
        y[b] += yp.reshape(SB, NSB, D).transpose(1, 0, 2).reshape(S, D)
    y += out_b[None, None, :]
    return y
